# revision 21
# baseline (speedup 1.0000x reference)
"""Trainium2 Bass kernel for MinimalKAN forward (nn_MinimalKAN_Normalized).

Math:
  a = sigmoid(alpha)
  out = (1-a) * (x @ W.T + b) + (a/sqrt(I)) * (x @ C0 + x^2 @ C1 + x^3 @ C2)

Folding the alpha blend into the weights on the host gives exactly
  out = x @ A + x^2 @ B + x^3 @ C + b_eff
with A = (1-a) W.T + s C0, B = s C1, C = s C2, b_eff = (1-a) b, s = a/sqrt(I).

Device strategy (data-parallel over batch, 8 cores), per core 4096 rows.
The contraction index i sits on SBUF partitions; the host feeds x^T in fp16.
Mixed precision split by term magnitude (fp8 for the linear term fails the
2e-2 gate: measured 3.1e-2 all-fp8, 2.3e-2 half-fp8):
  - linear term x @ A: fp16 matmuls, 4 accumulating matmuls per 128-row tile.
  - kan terms x^2 @ B + x^3 @ C: fp8(e4m3) DoubleRow matmuls (2 k-planes
    per instruction).
Both weight sets share host scale S=4096 (|A|*4096 < ~90, fp16-safe), so
linear and kan matmuls accumulate into ONE PSUM bank per tile and a single
DVE scalar_tensor_tensor does merge+descale+bias: out = psum/4096 + b (fp16).

PE schedule: flat software pipeline over the 32 tiles, 2-tile stagger --
tile t's four fp16 matmuls interleave one-for-one with tile (t-2)'s four DR
matmuls.  A DR LDWEIGHTS does not hide under a preceding DR matmul (+187ns
measured on DR-after-DR) but hides under an fp16 matmul's 512-cycle stream.
Measured steady state ~447ns per (f16,DR) pair = the N=512 issue roofline
(f16 216ns + DR stream ~231ns).

Head: the PE queue is strictly in-order, so emission order must match DMA
arrival order or a waiting matmul head-of-line-blocks ready ones.  Arrival
order (two HWDGE rings, ~110-200 GB/s/queue while all 8 cores fill;
pre-TileContext weight kicks were tried and hurt: the kick generation
delays the context entry barrier and the early weight bytes starve x01):
  scalar: wl01 | wk(x^2) | wk(x^3) | ...outputs
  sync:   x01 | wl23 | x23 | bias | x45 | x67 | x[g2] | ...
Emission: warmup dummies (PE busy from ~7.2us so the HAM clock gate is
released before real work arrives; short data stalls later don't re-throttle),
{t0,t1}x{k0,k1}, {t0,t1}x{k2,k3}, then the flat pipeline from t=2
(lin t, DR t-2).  Groups 0 and 1 build the
basis per-tile (ACT square -> GpSimd cube); later groups per-group.
Tail: last group drains per-tile, final tile in two 256-col halves.
"""

import os
import numpy as np

import concourse.bass as bass
from concourse import bacc
import concourse.mybir as mybir
import concourse.tile as tile
from concourse.bass_utils import run_bass_kernel_spmd

N_CORES = 8
B, I, O = 32768, 512, 512
BS = B // N_CORES          # rows per core
P = 128
KS = I // P                # 4 contraction k-tiles per basis
N_TILES = BS // P          # 32 x 128-row tiles per core
G = 4                      # tiles per x^2/x^3 group
N_GROUPS = N_TILES // G
LAG = 2                    # DR matmuls trail fp16 matmuls by this many tiles

S = 4096.0                 # shared host weight scale (fp16 linear + fp8 kan)
N_WARM1 = int(os.environ.get("KAN_WARM1", "33"))


def _build(repeat: int = 1) -> bass.Bass:
    f16 = mybir.dt.float16
    f8 = mybir.dt.float8e4
    f32 = mybir.dt.float32
    sq = mybir.ActivationFunctionType.Square
    DR = mybir.MatmulPerfMode.DoubleRow
    mult = mybir.AluOpType.mult
    add = mybir.AluOpType.add

    nc = bacc.Bacc("TRN2", target_bir_lowering=False, debug=False,
                   num_devices=N_CORES)

    x_d = nc.dram_tensor("xt", [P, N_TILES, KS, P], f16,
                         kind="ExternalInput")
    wl_d = nc.dram_tensor("wlin", [P, KS, O], f16, kind="ExternalInput")
    wk_d = nc.dram_tensor("wkan", [P, 2 * KS, O], f8, kind="ExternalInput")
    b_d = nc.dram_tensor("bias", [P, O], f16, kind="ExternalInput")
    o_d = nc.dram_tensor("out", [P, N_TILES, O], f16,
                         kind="ExternalOutput")

    with tile.TileContext(nc) as tc:
        with (
            tc.tile_pool(name="const", bufs=1) as const,
            tc.tile_pool(name="xin", bufs=3) as xin,
            tc.tile_pool(name="basis", bufs=3) as basis,
            tc.tile_pool(name="outp", bufs=3) as outp,
            tc.tile_pool(name="ps", bufs=6, space="PSUM") as ps,
            tc.tile_pool(name="ps_w", bufs=1, space="PSUM") as ps_w,
        ):
            # scalar-ring kicks (arrival order = consumption order)
            wl_sb = const.tile([P, KS, O], f16)
            nc.scalar.dma_start(wl_sb[:, 0:2, :], wl_d[:, 0:2, :])
            wk_sb = const.tile([P, 2 * KS, O], f8)
            nc.scalar.dma_start(wk_sb[:, 0:KS, :], wk_d[:, 0:KS, :])
            nc.scalar.dma_start(wk_sb[:, KS:2 * KS, :], wk_d[:, KS:2 * KS, :])
            bsb = const.tile([P, O], f16)

            warm = const.tile([P, P], f16)
            nc.vector.memset(warm[:], 0.0)
            po_w = ps_w.tile([P, P], f32, tag="po_w")

            def warmup(n):
                for _ in range(n):
                    nc.tensor.matmul(po_w[:], warm[:], warm[:],
                                     start=True, stop=True,
                                     skip_group_check=True)

            for rep in range(repeat):
                xts = {}
                b8s = {}
                osbs = {}
                banks = {}

                def group_input(g):
                    xT = xin.tile([P, G, KS, P], f16, name=f"xT_{rep}_{g}",
                                  tag="xT")
                    xts[g] = xT
                    if g == 0:
                        # sync-ring arrival order: x01, wl2, wl3, x2, x3,
                        # bias -- 128KB slices so each consumer unblocks
                        # as its own slice lands (in-order PE queue)
                        nc.sync.dma_start(xT[:, 0:2], x_d[:, 0:2])
                        nc.sync.dma_start(wl_sb[:, 2:3, :], wl_d[:, 2:3, :])
                        nc.sync.dma_start(wl_sb[:, 3:4, :], wl_d[:, 3:4, :])
                        nc.sync.dma_start(xT[:, 2:3], x_d[:, 2:3])
                        nc.sync.dma_start(xT[:, 3:4], x_d[:, 3:4])
                        nc.sync.dma_start(bsb[:], b_d[:])
                    elif g == 1:
                        nc.sync.dma_start(xT[:, 0:2], x_d[:, G:G + 2])
                        nc.sync.dma_start(xT[:, 2:4], x_d[:, G + 2:G + 4])
                    else:
                        nc.sync.dma_start(xT[:], x_d[:, g * G:(g + 1) * G])
                    b8 = basis.tile([P, G, 2 * KS, P], f8,
                                    name=f"b8_{rep}_{g}", tag="b8")
                    b8s[g] = b8
                    if g <= 1:
                        # per-tile basis so the first DR matmuls don't wait
                        # on the whole group's square/cube
                        for j in range(G):
                            nc.scalar.activation(b8[:, j, 0:KS, :],
                                                 xT[:, j], sq)
                            nc.gpsimd.tensor_mul(b8[:, j, KS:2 * KS, :],
                                                 b8[:, j, 0:KS, :],
                                                 xT[:, j])
                    else:
                        nc.scalar.activation(b8[:, :, 0:KS, :], xT[:], sq)
                        nc.gpsimd.tensor_mul(b8[:, :, KS:2 * KS, :],
                                             b8[:, :, 0:KS, :], xT[:])
                    osbs[g] = outp.tile([P, G, O], f16,
                                        name=f"o_{rep}_{g}", tag="o_sb")

                LT = N_TILES - 1    # final tile: STT + drain in two
                H = O // 2          # 256-col halves to shorten the tail

                def lin_mm(t, k):
                    g, j = divmod(t, G)
                    nc.tensor.matmul(
                        banks[t][:], xts[g][:, j, k, :], wl_sb[:, k, :],
                        start=(k == 0), stop=False, skip_group_check=True)

                def dr_mm(t, k):
                    g, j = divmod(t, G)
                    nc.tensor.matmul(
                        banks[t][:], b8s[g][:, j, 2 * k:2 * k + 2, :],
                        wk_sb[:, 2 * k:2 * k + 2, :],
                        start=False, stop=(k == KS - 1),
                        perf_mode=DR, skip_group_check=True)

                # ---- head: warmups + tiles 0,1 in DMA-arrival order ----
                group_input(0)
                banks[0] = ps.tile([P, O], f32, name=f"po_{rep}_0", tag="po")
                banks[1] = ps.tile([P, O], f32, name=f"po_{rep}_1", tag="po")
                warmup(N_WARM1)
                for t in (0, 1):
                    for k in (0, 1):
                        lin_mm(t, k)
                for t in (0, 1):
                    for k in (2, 3):
                        lin_mm(t, k)

                # ---- flat pipeline: lin tile t, DR tile t-LAG ----
                for t in range(LAG, N_TILES + LAG):
                    lt = t if t < N_TILES else -1
                    dt_ = t - LAG
                    if lt >= 0:
                        gl, jl = divmod(lt, G)
                        if jl == 0:
                            group_input(gl)
                        banks[lt] = ps.tile([P, O], f32,
                                            name=f"po_{rep}_{lt}", tag="po")
                    for k in range(KS):
                        if lt >= 0:
                            lin_mm(lt, k)
                        dr_mm(dt_, k)
                    gd, jd = divmod(dt_, G)
                    nc.vector.scalar_tensor_tensor(
                        osbs[gd][:, jd, :], banks.pop(dt_)[:], 1.0 / S,
                        bsb[:], mult, add)
                    if dt_ == LT:
                        # final tile drains in halves on BOTH rings so the
                        # two 64KB flights run in parallel
                        nc.sync.dma_start(
                            o_d[:, dt_, 0:H], osbs[gd][:, jd, 0:H])
                        nc.scalar.dma_start(
                            o_d[:, dt_, H:O], osbs[gd][:, jd, H:O])
                    elif gd == N_GROUPS - 1:
                        nc.scalar.dma_start(
                            o_d[:, dt_, :], osbs[gd][:, jd, :])
                    elif jd == G - 1:
                        nc.scalar.dma_start(
                            o_d[:, gd * G:(gd + 1) * G, :], osbs[gd][:])

    nc.compile()
    return nc


_NC_CACHE: dict[int, bass.Bass] = {}


def _get_nc(repeat: int = 1) -> bass.Bass:
    nc = _NC_CACHE.get(repeat)
    if nc is None:
        nc = _build(repeat)
        _NC_CACHE[repeat] = nc
    return nc


def _fold_weights(coeffs, W, b, alpha):
    a = 1.0 / (1.0 + np.exp(-np.float64(alpha)))
    s = a / np.sqrt(np.float64(I))
    A = (1.0 - a) * W.astype(np.float64).T + s * coeffs[:, :, 0].astype(np.float64)
    Bm = s * coeffs[:, :, 1].astype(np.float64)
    Cm = s * coeffs[:, :, 2].astype(np.float64)
    # [I, O] -> [P, KS, O] with row ks*P+p on partition p, slot ks
    wlin = (A * S).astype(np.float16)
    wlin = np.ascontiguousarray(
        wlin.reshape(KS, P, O).transpose(1, 0, 2))
    f8np = mybir.dt.np(mybir.dt.float8e4)
    wkan = np.concatenate([Bm * S, Cm * S], axis=0)
    wkan = np.clip(wkan, -240.0, 240.0).astype(f8np)
    wkan = np.ascontiguousarray(
        wkan.reshape(2 * KS, P, O).transpose(1, 0, 2))
    b_eff = ((1.0 - a) * b.astype(np.float64)).astype(np.float16)
    bias_rep = np.ascontiguousarray(
        np.broadcast_to(b_eff[None, :], (P, O)))
    return wlin, wkan, bias_rep


def _make_in_maps(x, coeffs, W, b, alpha):
    wlin, wkan, bias_rep = _fold_weights(coeffs, W, b, alpha)
    x = np.asarray(x, dtype=np.float32)
    in_maps = []
    for c in range(N_CORES):
        shard = x[c * BS:(c + 1) * BS].astype(np.float16)
        # [BS, I] -> [P, N_TILES, KS, P]: xt[p, t, ks, c'] =
        # x[t*P+c', ks*P+p]
        xt = np.ascontiguousarray(
            shard.reshape(N_TILES, P, KS, P).transpose(3, 0, 2, 1))
        in_maps.append({
            "wlin": wlin, "wkan": wkan, "bias": bias_rep, "xt": xt,
        })
    return in_maps


def _unpack_out(raw):
    # [P, N_TILES, O] fp16 -> [BS, O] f32: row t*P + p
    return np.ascontiguousarray(
        np.asarray(raw).astype(np.float32).transpose(1, 0, 2)
    ).reshape(BS, O)


def _run(x, coeffs, W, b, alpha, trace=False):
    nc = _get_nc()
    in_maps = _make_in_maps(x, coeffs, W, b, alpha)
    res = run_bass_kernel_spmd(nc, in_maps, core_ids=list(range(N_CORES)),
                               trace=trace)
    out = np.concatenate([_unpack_out(r["out"]) for r in res.results], axis=0)
    return out, res


def kernel(x, coeffs, W, b, alpha):
    out, _ = _run(x, coeffs, W, b, alpha, trace=False)
    return out
